# revision 44
# baseline (speedup 1.0000x reference)
"""Trainium2 Bass kernel for AttentionSimple (linear/kernelized attention).

Computes, for x:[B,N,C], w_qkv:[C,3C], w_proj:[C,C], b_proj:[C]:
    qkv = x @ w_qkv -> split q,k,v per head (H=12, D=64)
    kv  = (k^T v) * D^-0.5          per (b, h)     [D, D]
    out = gelu(q) @ gelu(kv)        per (b, h)     [N, D]
    y   = out @ w_proj + b_proj

Sharding: data-parallel over batch B=16 across 8 NeuronCores (2 batches/core).
All matmuls run in bf16 with fp32 PSUM accumulation.

Algorithm (FLOP-reduced):
  * kv goes through the Gram matrix G = x^T x (symmetric: only the upper
    block-triangle is computed; lower blocks are PE-transposed mirrors),
    then A = G @ W_k and kv^T_pair = W_v_pair^T @ A_pair -- swapping the
    Wk/Wv roles around symmetric G directly yields kv^T, which is the
    orientation the M-build wants as a stationary operand.
  * the per-head attention matmul is folded into the projection:
    y = gelu(q) @ M + b  with  M = blockdiag_h(gelu(kv_h)) @ w_proj,
    eliminating the whole attention pass and its PSUM evacuations.

Schedule per core (per batch b):
  boot:    zero-scratch HAM-warmup matmuls bridge the first x DMA wait.
  pass 1a: x slices stream (SWDGE fp32->bf16) into a persistent full-batch
           x_all tile; per 128-token chunk: PE transposes build x^T slices
           (batched strided DVE evacuation) and G rows 0-2 accumulate in
           PSUM; q^T chunks (lhsT = w_q chunk, rhs = x^T, gelu fused into
           the ACT evacuation) run two slices behind the transposes so the
           initial w_q DMA latency stays hidden.
  pass 1b: G rows 3-5 read the SBUF-resident x_all (no HBM reload); mirror
           transposes of finished G rows interleave on a dedicated psum
           bank; leftover q units drain here.
  A/kv/M:  A = G @ W_k (evacs split DVE/ACT at the bank boundary); per-pair
           kv^T accumulates in its own rotating psum bank with the gelu
           issued right after that pair's last matmul; M built per pair
           from the block-diagonal gelu(kv^T) tiles; remaining q units
           fill the dependency bubbles.
  pass 2:  y = gqT^T @ M + b as 512/256-wide matmuls into a 4-deep psum
           pool; bias added on DVE; output DMAs alternate the two HWDGE
           rings (SP/ACT); next batch's x prefetches underneath.

Self-contained: hardcodes shapes; builds the Bass program, runs it SPMD on
cores 0-7 via bass_utils.run_bass_kernel_spmd, returns the gathered output.
"""

import numpy as np

import concourse.bacc as bacc
import concourse.bass as bass
import concourse.mybir as mybir
import concourse.tile as tile
from concourse import masks
from concourse.bass_utils import run_bass_kernel_spmd

F32 = mybir.dt.float32
BF16 = mybir.dt.bfloat16
GELU = mybir.ActivationFunctionType.Gelu
COPY = mybir.ActivationFunctionType.Copy
PSUM = bass.MemorySpace.PSUM

B, N, C = 16, 4096, 768
H, D = 12, 64
SCALE = D**-0.5
NCORES = 8
BPC = B // NCORES  # batches per core
CCH = C // 128  # 6 column chunks of 128
NTS = N // 512  # 8 slices of 512 tokens
NPAIR = H // 2  # 6 head pairs (128 cols each)


def _build_program():
    nc = bacc.Bacc("TRN2", target_bir_lowering=False, debug=False)

    x_d = nc.dram_tensor("x", [BPC, N, C], F32, kind="ExternalInput").ap()
    wq_d = nc.dram_tensor("w_qkv", [C, 3 * C], F32, kind="ExternalInput").ap()
    wp_d = nc.dram_tensor("w_proj", [C, C], F32, kind="ExternalInput").ap()
    bp_d = nc.dram_tensor("b_proj", [C], F32, kind="ExternalInput").ap()
    y_d = nc.dram_tensor("y", [BPC, N, C], F32, kind="ExternalOutput").ap()

    with tile.TileContext(nc) as tc:
        with (
            tc.tile_pool(name="weights", bufs=1) as wpool,
            tc.tile_pool(name="acts", bufs=1) as apool,
            tc.tile_pool(name="xin", bufs=1) as xapool,
            tc.tile_pool(name="xt", bufs=4) as xtpool,
            tc.tile_pool(name="yout", bufs=5) as ypool,
            tc.tile_pool(name="ps_mm", bufs=3, space=PSUM) as ps_mm,
        ):
            # ---- boot order matters: gpsimd executes in FIFO order, so the
            # tiny scratch memset goes first (unblocks the PE warmup), then
            # the first x slice, then the identity, then the second slice.
            ident = wpool.tile([128, 128], BF16)
            scratch = wpool.tile([128, 128], BF16)
            nc.gpsimd.memset(scratch[:], 0.0)

            # ---- full per-batch x in bf16, [c on part? no: t-chunks, 768]:
            # x_all[p, tch, c] = x[b, tch*128+p, c].  Loaded once per batch
            # (no pass-1b reload); both G passes + transposes read it.
            x_all_tiles = {}

            def get_x_all(b):
                if b not in x_all_tiles:
                    x_all_tiles[b] = xapool.tile(
                        [128, N // 128, C], BF16, tag="x_all", name=f"x_all{b}"
                    )
                return x_all_tiles[b]

            # ---- prefetch the first token slice before the big weight DMAs
            # so the SWDGE rings deliver x(0,0) immediately and the PE can
            # start transposing while weights stream in.
            def load_x(b, ts):
                xa = get_x_all(b)
                for tc4 in range(4):
                    t0 = ts * 512 + tc4 * 128
                    nc.gpsimd.dma_start(
                        xa[:, ts * 4 + tc4, :], x_d[b, t0 : t0 + 128, :]
                    )

            load_x(0, 0)
            masks.make_identity(nc, ident[:])
            load_x(0, 1)  # second slice before weights: slot 1 needs it early

            # ---- HAM warmup: throwaway matmuls on the zero scratch bridge
            # the x(0,0) DMA wait and flip the PE clock gate to 8/8 before
            # real work starts
            with tc.tile_pool(name="ps_warm", bufs=1, space=PSUM) as ps_warm:
                warm = ps_warm.tile([128, 128], F32)
                for _ in range(40):
                    nc.tensor.matmul(warm[:], scratch[:], scratch[:])

            # ---- weights: q 512-col slices first (needed first), then k/v,
            # proj ----
            w_qkv = wpool.tile([128, CCH, 3 * C], BF16)  # 27.6KB/part
            w_proj = wpool.tile([128, CCH, C], BF16)  # 9.2KB/part
            for lo, hi in ((0, 512), (512, 768)):  # q slices (needed first)
                for cch in range(CCH):
                    nc.gpsimd.dma_start(
                        w_qkv[:, cch, lo:hi], wq_d[cch * 128 : (cch + 1) * 128, lo:hi]
                    )
            # Remaining weight loads are deferred into the first batch's
            # ts-loop (3 per slice) so their issue cost and ring bandwidth
            # don't delay the x-tile prefetches. Deps still order correctly.
            b_bc = wpool.tile([128, C], F32)
            deferred_w = []
            for cch in range(CCH):  # k part (A-stage), then v part (kv-stage)
                deferred_w.append(
                    (
                        w_qkv[:, cch, C : 2 * C],
                        wq_d[cch * 128 : (cch + 1) * 128, C : 2 * C],
                    )
                )
            for cch in range(CCH):
                deferred_w.append(
                    (w_qkv[:, cch, 2 * C :], wq_d[cch * 128 : (cch + 1) * 128, 2 * C :])
                )
            for cch in range(CCH):
                deferred_w.append(
                    (w_proj[:, cch, :], wp_d[cch * 128 : (cch + 1) * 128, :])
                )
            deferred_w.append((b_bc[:], bp_d.unsqueeze(0).partition_broadcast(128)))
            deferred_w.reverse()

            # gkv: per-pair block-diagonal [128,128] with gelu(kv^T*scale) of
            # the even head at [0:64,0:64] and odd head at [64:,64:].  One
            # buffer for all batches: the off-diagonal zeros are written once
            # here and never touched again (gelus only write the diagonals).
            gkv = apool.tile([128, NPAIR, 128], BF16, tag="gkv")
            nc.gpsimd.memset(gkv[:], 0.0)

            for b in range(BPC):
                # gqT: q^T with gelu applied, [c=768, t=4096] as 6 chunks
                gqT = apool.tile([128, CCH, N], BF16, tag="gqT")
                # M: folded projection, rows pair-chunked: M = blkdiag(gkv)@Wp
                M_sb = apool.tile([128, NPAIR, C], BF16, tag="M")
                # Gram matrix G = x^T x (bf16), used for kv = W_k^T G W_v
                G_sb = apool.tile([128, CCH, C], BF16, tag="G")

                x_all = get_x_all(b)

                # q-chunk units (ts, jch) are emitted 2 token-slices behind
                # the transposes that produce their xT input: q(ts) runs in
                # slot ts+2.  This hides the initial w_q DMA latency behind
                # two slots of transpose+G work; the ~12 units left over at
                # the end of pass 1a are drained in pass 1b and in the
                # A/kv/M dependency bubbles.
                xT_slices = {}

                def q_unit(ts, jch):
                    xTs = xT_slices[ts]
                    pq = ps_mm.tile([128, 512], F32, tag="pmm", name="pq")
                    for cch in range(CCH):
                        nc.tensor.matmul(
                            pq[:],
                            w_qkv[:, cch, jch * 128 : (jch + 1) * 128],
                            xTs[:, cch, 0:512],
                            start=(cch == 0),
                            stop=(cch == CCH - 1),
                        )
                    nc.scalar.activation(
                        gqT[:, jch, ts * 512 : ts * 512 + 512], pq[:], GELU
                    )

                q_iter = iter(
                    [(ts, jch) for ts in range(NTS) for jch in range(CCH)]
                )

                def pop_q(n):
                    for _ in range(n):
                        tj = next(q_iter, None)
                        if tj is None:
                            return
                        q_unit(*tj)

                # ===== pass 1a: q^T + upper-triangular G rows 0..2 ==========
                with tc.tile_pool(name="ps_gA", bufs=1, space=PSUM) as ps_gA:
                    g_acc = [
                        ps_gA.tile([128, C - ci * 128], F32, tag=f"g{ci}", name=f"gA{ci}")
                        for ci in range(3)
                    ]
                    for ts in range(NTS):
                        xT = xtpool.tile([128, CCH, 512], BF16)
                        xT_slices[ts] = xT
                        # slices 0,1 of each batch are prefetched earlier, so
                        # slot ts tops up slice ts+2
                        if ts + 2 <= NTS - 1:
                            load_x(b, ts + 2)
                        elif b + 1 < BPC:
                            load_x(b + 1, ts + 2 - NTS)
                        for _ in range(3):
                            if deferred_w:
                                dst, srcap = deferred_w.pop()
                                nc.gpsimd.dma_start(dst, srcap)
                        # q of slice ts-2 goes FIRST: its inputs (xT, w_q) are
                        # ready, so it must not sit behind this slot's
                        # transposes in the in-order PE queue when the x DMA
                        # is late (startup).
                        if ts >= 2:
                            pop_q(CCH)
                        for tc4 in range(4):
                            x_bf = x_all[:, ts * 4 + tc4, :]
                            tr = ps_mm.tile([128, CCH * 128], BF16, tag="pmm")
                            for cch in range(CCH):
                                nc.tensor.transpose(
                                    tr[:, cch * 128 : (cch + 1) * 128],
                                    x_bf[:, cch * 128 : (cch + 1) * 128],
                                    ident[:],
                                )
                            nc.vector.tensor_copy(
                                xT[:, :, tc4 * 128 : tc4 * 128 + 128],
                                tr[:].rearrange("p (c f) -> p c f", c=CCH),
                            )
                            # G rows ci, cols [ci*128:768), accumulated over
                            # all 32 token chunks; each 512/256-col split owns
                            # its psum bank so start=(first chunk) is safe.
                            first = ts == 0 and tc4 == 0
                            last = ts == NTS - 1 and tc4 == 3
                            for ci in range(3):
                                w = C - ci * 128
                                for lo in range(0, w, 512):
                                    hi = min(lo + 512, w)
                                    nc.tensor.matmul(
                                        g_acc[ci][:, lo:hi],
                                        x_bf[:, ci * 128 : (ci + 1) * 128],
                                        x_bf[:, ci * 128 + lo : ci * 128 + hi],
                                        start=first,
                                        stop=last,
                                        skip_group_check=True,
                                    )
                    for ci in range(3):
                        nc.vector.tensor_copy(
                            G_sb[:, ci, ci * 128 : C], g_acc[ci][:]
                        )

                # ===== pass 1b: G rows 3..5 (x cols 384: from SBUF) =========
                early_mirrors = [
                    (1, 0), (2, 0), (2, 1), (3, 0), (3, 1), (3, 2),
                    (4, 0), (4, 1), (4, 2), (5, 0), (5, 1), (5, 2),
                ][::-1]
                with (
                    tc.tile_pool(name="ps_gB", bufs=1, space=PSUM) as ps_gB,
                    tc.tile_pool(name="ps_pt", bufs=1, space=PSUM) as ps_pt,
                ):
                    g_accB = [
                        ps_gB.tile([128, C - ci * 128], F32, tag=f"g{ci}", name=f"gB{ci}")
                        for ci in range(3, CCH)
                    ]

                    for ts in range(NTS):
                        pop_q(1)
                        # fill pass 1b with mirror transposes whose sources
                        # (G rows 0-2) were finished in pass 1a; they use a
                        # dedicated psum bank so they don't contend with the
                        # q units' ps_mm rotation
                        for _ in range(2):
                            if early_mirrors:
                                i, j = early_mirrors.pop()
                                pt = ps_pt.tile(
                                    [128, 128], BF16, tag="pt", name=f"pt{i}{j}"
                                )
                                nc.tensor.transpose(
                                    pt[:],
                                    G_sb[:, j, i * 128 : i * 128 + 128],
                                    ident[:],
                                )
                                nc.vector.tensor_copy(
                                    G_sb[:, i, j * 128 : j * 128 + 128], pt[:]
                                )
                        for tc4 in range(4):
                            x_hf = x_all[:, ts * 4 + tc4, 384:C]
                            first = ts == 0 and tc4 == 0
                            last = ts == NTS - 1 and tc4 == 3
                            for k, ci in enumerate(range(3, CCH)):
                                off = ci * 128 - 384
                                nc.tensor.matmul(
                                    g_accB[k][:],
                                    x_hf[:, off : off + 128],
                                    x_hf[:, off:],
                                    start=first,
                                    stop=last,
                                    skip_group_check=True,
                                )
                    # parallelize the tail evacs across DVE and ACT so the
                    # late mirrors (which need rows 3-5) start sooner
                    nc.vector.tensor_copy(G_sb[:, 3, 384:C], g_accB[0][:])
                    nc.scalar.activation(G_sb[:, 4, 512:C], g_accB[1][:], COPY)
                    nc.scalar.activation(G_sb[:, 5, 640:C], g_accB[2][:], COPY)
                for n, (i, j) in enumerate(((4, 3), (5, 3), (5, 4))):
                    pt = ps_mm.tile([128, 128], BF16, tag="pmm", name=f"pt{i}{j}")
                    nc.tensor.transpose(
                        pt[:], G_sb[:, j, i * 128 : i * 128 + 128], ident[:]
                    )
                    if n % 2 == 0:
                        nc.vector.tensor_copy(
                            G_sb[:, i, j * 128 : j * 128 + 128], pt[:]
                        )
                    else:
                        nc.scalar.activation(
                            G_sb[:, i, j * 128 : j * 128 + 128], pt[:], COPY
                        )


                # ---- A = G @ W_k  (contraction over c) ----
                A_sb = apool.tile([128, CCH, C], BF16, tag="A")
                with tc.tile_pool(name="ps_A", bufs=2, space=PSUM) as ps_A:
                    for cp in range(CCH):
                        pA = ps_A.tile([128, C], F32, tag="pA")
                        for lo, hi in ((0, 512), (512, 768)):
                            for cch in range(CCH):
                                nc.tensor.matmul(
                                    pA[:, lo:hi],
                                    G_sb[:, cch, cp * 128 : (cp + 1) * 128],
                                    w_qkv[:, cch, C + lo : C + hi],
                                    start=(cch == 0),
                                    stop=(cch == CCH - 1),
                                    skip_group_check=True,
                                )
                        # split each evac at the psum bank boundary so DVE and
                        # ACT read different banks in parallel and the 2 psum
                        # bufs recycle fast enough to keep the A matmuls dense
                        nc.vector.tensor_copy(A_sb[:, cp, 0:512], pA[:, 0:512])
                        nc.scalar.activation(
                            A_sb[:, cp, 512:768], pA[:, 512:768], COPY
                        )
                    pop_q(2)  # fill the A-evac -> kv dependency bubble

                # ---- kv^T pairs = W_v_pair^T @ A_pair = (W_k^T G W_v)^T,
                # then gelu(kv^T * scale) into block-diag pair tiles ----
                # each pair's kv^T accumulates in its own (bank-aligned) psum
                # buffer so its gelu can read while the PE writes later pairs
                # in other banks (PE-W + ACT-R same bank is a HW error)
                with tc.tile_pool(name="ps_kv", bufs=4, space=PSUM) as ps_kv:
                    for pr in range(NPAIR):
                        kv_pr = ps_kv.tile([128, 512], F32, tag="kv", name="kv_pr")
                        for cch in range(CCH):
                            nc.tensor.matmul(
                                kv_pr[:, 0:128],
                                w_qkv[:, cch, 2 * C + pr * 128 : 2 * C + (pr + 1) * 128],
                                A_sb[:, cch, pr * 128 : (pr + 1) * 128],
                                start=(cch == 0),
                                stop=(cch == CCH - 1),
                                skip_group_check=True,
                            )
                        # gelu right after this pair's stop so ACT chews
                        # through the 12 small gelus while PE does later pairs
                        nc.scalar.activation(
                            gkv[0:64, pr, 0:64],
                            kv_pr[0:64, 0:64],
                            GELU,
                            scale=SCALE,
                        )
                        nc.scalar.activation(
                            gkv[64:128, pr, 64:128],
                            kv_pr[64:128, 64:128],
                            GELU,
                            scale=SCALE,
                        )

                    # ---- M = blockdiag_h(gelu(kv_h)) @ w_proj  (per pair):
                    # lhsT = gelu(kv^T) blockdiag so lhsT^T = gelu(kv) blockdiag
                    for pr in range(NPAIR):
                        pM0 = ps_mm.tile([128, 512], F32, tag="pmm")
                        pM1 = ps_mm.tile([128, 256], F32, tag="pmm")
                        nc.tensor.matmul(
                            pM0[:], gkv[:, pr, :], w_proj[:, pr, 0:512],
                            skip_group_check=True,
                        )
                        nc.tensor.matmul(
                            pM1[:], gkv[:, pr, :], w_proj[:, pr, 512:768],
                            skip_group_check=True,
                        )
                        # DVE-heavy evac split: ACT is busy with the gelus
                        nc.vector.tensor_copy(M_sb[:, pr, 0:512], pM0[:])
                        if pr % 2 == 0:
                            nc.vector.tensor_copy(M_sb[:, pr, 512:768], pM1[:])
                        else:
                            nc.scalar.activation(M_sb[:, pr, 512:768], pM1[:], COPY)
                    pop_q(2)  # fill the M-evac -> pass-2 dependency bubble

                # ============ pass 2: y = gelu(q) @ M + b  (single GEMM) ====
                with tc.tile_pool(name="ps_y", bufs=4, space=PSUM) as ps_y:
                  for ts in range(NTS):
                    for tc4 in range(4):
                        tsl = slice(
                            ts * 512 + tc4 * 128, ts * 512 + tc4 * 128 + 128
                        )
                        py0 = ps_y.tile([128, 512], F32, tag="py")
                        py1 = ps_y.tile([128, 256], F32, tag="py", name="py1")
                        for cch in range(CCH):
                            last = cch == CCH - 1
                            nc.tensor.matmul(
                                py0[:],
                                gqT[:, cch, tsl],
                                M_sb[:, cch, 0:512],
                                start=(cch == 0),
                                stop=last,
                                skip_group_check=True,
                            )
                            nc.tensor.matmul(
                                py1[:],
                                gqT[:, cch, tsl],
                                M_sb[:, cch, 512:768],
                                start=(cch == 0),
                                stop=last,
                                skip_group_check=True,
                            )
                        y_sb = ypool.tile([128, C], F32)
                        nc.vector.tensor_add(y_sb[:, 0:512], py0[:], b_bc[:, 0:512])
                        nc.vector.tensor_add(
                            y_sb[:, 512:768], py1[:], b_bc[:, 512:768]
                        )
                        t0 = ts * 512 + tc4 * 128
                        if b == BPC - 1 and ts == NTS - 1 and tc4 >= 2:
                            # drain the last tiles on both rings in halves so
                            # transfer+receipt latencies overlap at the tail
                            nc.sync.dma_start(
                                y_d[b, t0 : t0 + 128, 0:384], y_sb[:, 0:384]
                            )
                            nc.scalar.dma_start(
                                y_d[b, t0 : t0 + 128, 384:C], y_sb[:, 384:C]
                            )
                        else:
                            # alternate the two HWDGE rings (SP / ACT) so the
                            # output queue drains twice as fast
                            eng = nc.sync if tc4 % 2 == 0 else nc.scalar
                            eng.dma_start(y_d[b, t0 : t0 + 128, :], y_sb[:])

    nc.compile()
    return nc


_cached_nc = None


def kernel(x, w_qkv, w_proj, b_proj):
    global _cached_nc
    if _cached_nc is None:
        _cached_nc = _build_program()
    nc = _cached_nc

    x = np.ascontiguousarray(x, dtype=np.float32)
    in_maps = [
        {
            "x": x[i * BPC : (i + 1) * BPC],
            "w_qkv": np.asarray(w_qkv, dtype=np.float32),
            "w_proj": np.asarray(w_proj, dtype=np.float32),
            "b_proj": np.asarray(b_proj, dtype=np.float32),
        }
        for i in range(NCORES)
    ]
    last_err = None
    for _attempt in range(3):
        try:
            res = run_bass_kernel_spmd(nc, in_maps, core_ids=list(range(NCORES)))
            out = np.concatenate(
                [res.results[i]["y"] for i in range(NCORES)], axis=0
            )
            return out.astype(np.float32)
        except Exception as e:  # transient NRT device errors recover on retry
            last_err = e
    raise last_err



# revision 47
# speedup vs baseline: 1.0017x; 1.0017x over previous
"""Trainium2 Bass kernel for AttentionSimple (linear/kernelized attention).

Computes, for x:[B,N,C], w_qkv:[C,3C], w_proj:[C,C], b_proj:[C]:
    qkv = x @ w_qkv -> split q,k,v per head (H=12, D=64)
    kv  = (k^T v) * D^-0.5          per (b, h)     [D, D]
    out = gelu(q) @ gelu(kv)        per (b, h)     [N, D]
    y   = out @ w_proj + b_proj

Sharding: data-parallel over batch B=16 across 8 NeuronCores (2 batches/core).
All matmuls run in bf16 with fp32 PSUM accumulation.

Algorithm (FLOP-reduced):
  * kv goes through the Gram matrix G = x^T x (symmetric: only the upper
    block-triangle is computed; lower blocks are PE-transposed mirrors),
    then A = G @ W_k and kv^T_pair = W_v_pair^T @ A_pair -- swapping the
    Wk/Wv roles around symmetric G directly yields kv^T, which is the
    orientation the M-build wants as a stationary operand.
  * the per-head attention matmul is folded into the projection:
    y = gelu(q) @ M + b  with  M = blockdiag_h(gelu(kv_h)) @ w_proj,
    eliminating the whole attention pass and its PSUM evacuations.

Schedule per core (per batch b):
  boot:    zero-scratch HAM-warmup matmuls bridge the first x DMA wait.
  pass 1a: x slices stream (SWDGE fp32->bf16) into a persistent full-batch
           x_all tile; per 128-token chunk: PE transposes build x^T slices
           (batched strided DVE evacuation) and G rows 0-2 accumulate in
           PSUM; q^T chunks (lhsT = w_q chunk, rhs = x^T, gelu fused into
           the ACT evacuation) run two slices behind the transposes so the
           initial w_q DMA latency stays hidden.
  pass 1b: G rows 3-5 read the SBUF-resident x_all (no HBM reload); mirror
           transposes of finished G rows interleave on a dedicated psum
           bank; leftover q units drain here.
  A/kv/M:  A = G @ W_k (evacs split DVE/ACT at the bank boundary); per-pair
           kv^T accumulates in its own rotating psum bank with the gelu
           issued right after that pair's last matmul; M built per pair
           from the block-diagonal gelu(kv^T) tiles; remaining q units
           fill the dependency bubbles.
  pass 2:  y = gqT^T @ M + b as 512/256-wide matmuls into a 4-deep psum
           pool; bias added on DVE; output DMAs alternate the two HWDGE
           rings (SP/ACT); next batch's x prefetches underneath.

Self-contained: hardcodes shapes; builds the Bass program, runs it SPMD on
cores 0-7 via bass_utils.run_bass_kernel_spmd, returns the gathered output.
"""

import numpy as np

import concourse.bacc as bacc
import concourse.bass as bass
import concourse.mybir as mybir
import concourse.tile as tile
from concourse import masks
from concourse.bass_utils import run_bass_kernel_spmd

F32 = mybir.dt.float32
BF16 = mybir.dt.bfloat16
GELU = mybir.ActivationFunctionType.Gelu
COPY = mybir.ActivationFunctionType.Copy
PSUM = bass.MemorySpace.PSUM

B, N, C = 16, 4096, 768
H, D = 12, 64
SCALE = D**-0.5
NCORES = 8
BPC = B // NCORES  # batches per core
CCH = C // 128  # 6 column chunks of 128
NTS = N // 512  # 8 slices of 512 tokens
NPAIR = H // 2  # 6 head pairs (128 cols each)


def _build_program():
    nc = bacc.Bacc("TRN2", target_bir_lowering=False, debug=False)

    x_d = nc.dram_tensor("x", [BPC, N, C], F32, kind="ExternalInput").ap()
    wq_d = nc.dram_tensor("w_qkv", [C, 3 * C], F32, kind="ExternalInput").ap()
    wp_d = nc.dram_tensor("w_proj", [C, C], F32, kind="ExternalInput").ap()
    bp_d = nc.dram_tensor("b_proj", [C], F32, kind="ExternalInput").ap()
    y_d = nc.dram_tensor("y", [BPC, N, C], F32, kind="ExternalOutput").ap()

    with tile.TileContext(nc) as tc:
        with (
            tc.tile_pool(name="weights", bufs=1) as wpool,
            tc.tile_pool(name="acts", bufs=1) as apool,
            tc.tile_pool(name="xin", bufs=1) as xapool,
            tc.tile_pool(name="xt", bufs=4) as xtpool,
            tc.tile_pool(name="yout", bufs=5) as ypool,
            tc.tile_pool(name="ps_mm", bufs=3, space=PSUM) as ps_mm,
        ):
            # ---- boot order matters: gpsimd executes in FIFO order, so the
            # tiny scratch memset goes first (unblocks the PE warmup), then
            # the first x slice, then the identity, then the second slice.
            ident = wpool.tile([128, 128], BF16)
            scratch = wpool.tile([128, 128], BF16)
            nc.gpsimd.memset(scratch[:], 0.0)

            # ---- full per-batch x in bf16, [c on part? no: t-chunks, 768]:
            # x_all[p, tch, c] = x[b, tch*128+p, c].  Loaded once per batch
            # (no pass-1b reload); both G passes + transposes read it.
            x_all_tiles = {}

            def get_x_all(b):
                if b not in x_all_tiles:
                    x_all_tiles[b] = xapool.tile(
                        [128, N // 128, C], BF16, tag="x_all", name=f"x_all{b}"
                    )
                return x_all_tiles[b]

            # ---- prefetch the first token slice before the big weight DMAs
            # so the SWDGE rings deliver x(0,0) immediately and the PE can
            # start transposing while weights stream in.
            def load_x(b, ts):
                xa = get_x_all(b)
                for tc4 in range(4):
                    t0 = ts * 512 + tc4 * 128
                    nc.gpsimd.dma_start(
                        xa[:, ts * 4 + tc4, :], x_d[b, t0 : t0 + 128, :]
                    )

            load_x(0, 0)
            masks.make_identity(nc, ident[:])
            load_x(0, 1)  # second slice before weights: slot 1 needs it early

            # ---- HAM warmup: throwaway matmuls on the zero scratch bridge
            # the x(0,0) DMA wait and flip the PE clock gate to 8/8 before
            # real work starts
            with tc.tile_pool(name="ps_warm", bufs=1, space=PSUM) as ps_warm:
                warm = ps_warm.tile([128, 128], F32)
                for _ in range(40):
                    nc.tensor.matmul(warm[:], scratch[:], scratch[:])

            # ---- weights: q 512-col slices first (needed first), then k/v,
            # proj ----
            w_qkv = wpool.tile([128, CCH, 3 * C], BF16)  # 27.6KB/part
            w_proj = wpool.tile([128, CCH, C], BF16)  # 9.2KB/part
            for lo, hi in ((0, 512), (512, 768)):  # q slices (needed first)
                for cch in range(CCH):
                    nc.gpsimd.dma_start(
                        w_qkv[:, cch, lo:hi], wq_d[cch * 128 : (cch + 1) * 128, lo:hi]
                    )
            # Remaining weight loads are deferred into the first batch's
            # ts-loop (3 per slice) so their issue cost and ring bandwidth
            # don't delay the x-tile prefetches. Deps still order correctly.
            b_bc = wpool.tile([128, C], F32)
            deferred_w = []
            for cch in range(CCH):  # k part (A-stage), then v part (kv-stage)
                deferred_w.append(
                    (
                        w_qkv[:, cch, C : 2 * C],
                        wq_d[cch * 128 : (cch + 1) * 128, C : 2 * C],
                    )
                )
            for cch in range(CCH):
                deferred_w.append(
                    (w_qkv[:, cch, 2 * C :], wq_d[cch * 128 : (cch + 1) * 128, 2 * C :])
                )
            for cch in range(CCH):
                deferred_w.append(
                    (w_proj[:, cch, :], wp_d[cch * 128 : (cch + 1) * 128, :])
                )
            deferred_w.append((b_bc[:], bp_d.unsqueeze(0).partition_broadcast(128)))
            deferred_w.reverse()

            # gkv: per-pair block-diagonal [128,128] with gelu(kv^T*scale) of
            # the even head at [0:64,0:64] and odd head at [64:,64:].  One
            # buffer for all batches: the off-diagonal zeros are written once
            # here and never touched again (gelus only write the diagonals).
            gkv = apool.tile([128, NPAIR, 128], BF16, tag="gkv")
            nc.gpsimd.memset(gkv[:], 0.0)

            for b in range(BPC):
                # gqT: q^T with gelu applied, [c=768, t=4096] as 6 chunks
                gqT = apool.tile([128, CCH, N], BF16, tag="gqT")
                # M: folded projection, rows pair-chunked: M = blkdiag(gkv)@Wp
                M_sb = apool.tile([128, NPAIR, C], BF16, tag="M")
                # Gram matrix G = x^T x (bf16), used for kv = W_k^T G W_v
                G_sb = apool.tile([128, CCH, C], BF16, tag="G")

                x_all = get_x_all(b)

                # q-chunk units (ts, jch) are emitted 2 token-slices behind
                # the transposes that produce their xT input: q(ts) runs in
                # slot ts+2.  This hides the initial w_q DMA latency behind
                # two slots of transpose+G work; the ~12 units left over at
                # the end of pass 1a are drained in pass 1b and in the
                # A/kv/M dependency bubbles.
                xT_slices = {}

                def q_unit(ts, jch):
                    xTs = xT_slices[ts]
                    pq = ps_mm.tile([128, 512], F32, tag="pmm", name="pq")
                    for cch in range(CCH):
                        nc.tensor.matmul(
                            pq[:],
                            w_qkv[:, cch, jch * 128 : (jch + 1) * 128],
                            xTs[:, cch, 0:512],
                            start=(cch == 0),
                            stop=(cch == CCH - 1),
                        )
                    nc.scalar.activation(
                        gqT[:, jch, ts * 512 : ts * 512 + 512], pq[:], GELU
                    )

                q_iter = iter(
                    [(ts, jch) for ts in range(NTS) for jch in range(CCH)]
                )

                def pop_q(n):
                    for _ in range(n):
                        tj = next(q_iter, None)
                        if tj is None:
                            return
                        q_unit(*tj)

                # ===== pass 1a: q^T + upper-triangular G rows 0..2 ==========
                with tc.tile_pool(name="ps_gA", bufs=1, space=PSUM) as ps_gA:
                    g_acc = [
                        ps_gA.tile([128, C - ci * 128], F32, tag=f"g{ci}", name=f"gA{ci}")
                        for ci in range(3)
                    ]
                    for ts in range(NTS):
                        xT = xtpool.tile([128, CCH, 512], BF16)
                        xT_slices[ts] = xT
                        # slices 0,1 of each batch are prefetched earlier, so
                        # slot ts tops up slice ts+2
                        if ts + 2 <= NTS - 1:
                            load_x(b, ts + 2)
                        elif b + 1 < BPC:
                            load_x(b + 1, ts + 2 - NTS)
                        for _ in range(3):
                            if deferred_w:
                                dst, srcap = deferred_w.pop()
                                nc.gpsimd.dma_start(dst, srcap)
                        # q of slice ts-2 goes FIRST: its inputs (xT, w_q) are
                        # ready, so it must not sit behind this slot's
                        # transposes in the in-order PE queue when the x DMA
                        # is late (startup).
                        if ts >= 2:
                            pop_q(CCH)
                        for tc4 in range(4):
                            x_bf = x_all[:, ts * 4 + tc4, :]
                            tr = ps_mm.tile([128, CCH * 128], BF16, tag="pmm")
                            for cch in range(CCH):
                                nc.tensor.transpose(
                                    tr[:, cch * 128 : (cch + 1) * 128],
                                    x_bf[:, cch * 128 : (cch + 1) * 128],
                                    ident[:],
                                )
                            nc.vector.tensor_copy(
                                xT[:, :, tc4 * 128 : tc4 * 128 + 128],
                                tr[:].rearrange("p (c f) -> p c f", c=CCH),
                            )
                            # G rows ci, cols [ci*128:768), accumulated over
                            # all 32 token chunks; each 512/256-col split owns
                            # its psum bank so start=(first chunk) is safe.
                            first = ts == 0 and tc4 == 0
                            last = ts == NTS - 1 and tc4 == 3
                            for ci in range(3):
                                w = C - ci * 128
                                for lo in range(0, w, 512):
                                    hi = min(lo + 512, w)
                                    nc.tensor.matmul(
                                        g_acc[ci][:, lo:hi],
                                        x_bf[:, ci * 128 : (ci + 1) * 128],
                                        x_bf[:, ci * 128 + lo : ci * 128 + hi],
                                        start=first,
                                        stop=last,
                                        skip_group_check=True,
                                    )
                    for ci in range(3):
                        nc.vector.tensor_copy(
                            G_sb[:, ci, ci * 128 : C], g_acc[ci][:]
                        )

                # ===== pass 1b: G rows 3..5 (x cols 384: from SBUF) =========
                early_mirrors = [
                    (1, 0), (2, 0), (2, 1), (3, 0), (3, 1), (3, 2),
                    (4, 0), (4, 1), (4, 2), (5, 0), (5, 1), (5, 2),
                ][::-1]
                with (
                    tc.tile_pool(name="ps_gB", bufs=1, space=PSUM) as ps_gB,
                    tc.tile_pool(name="ps_pt", bufs=1, space=PSUM) as ps_pt,
                ):
                    g_accB = [
                        ps_gB.tile([128, C - ci * 128], F32, tag=f"g{ci}", name=f"gB{ci}")
                        for ci in range(3, CCH)
                    ]

                    for ts in range(NTS):
                        if ts > 0:
                            pop_q(1)
                        # fill pass 1b with mirror transposes whose sources
                        # (G rows 0-2) were finished in pass 1a; they use a
                        # dedicated psum bank so they don't contend with the
                        # q units' ps_mm rotation
                        for _ in range(2):
                            if early_mirrors:
                                i, j = early_mirrors.pop()
                                pt = ps_pt.tile(
                                    [128, 128], BF16, tag="pt", name=f"pt{i}{j}"
                                )
                                nc.tensor.transpose(
                                    pt[:],
                                    G_sb[:, j, i * 128 : i * 128 + 128],
                                    ident[:],
                                )
                                nc.vector.tensor_copy(
                                    G_sb[:, i, j * 128 : j * 128 + 128], pt[:]
                                )
                        for tc4 in range(4):
                            x_hf = x_all[:, ts * 4 + tc4, 384:C]
                            first = ts == 0 and tc4 == 0
                            last = ts == NTS - 1 and tc4 == 3
                            for k, ci in enumerate(range(3, CCH)):
                                off = ci * 128 - 384
                                nc.tensor.matmul(
                                    g_accB[k][:],
                                    x_hf[:, off : off + 128],
                                    x_hf[:, off:],
                                    start=first,
                                    stop=last,
                                    skip_group_check=True,
                                )
                    # parallelize the tail evacs across DVE and ACT so the
                    # late mirrors (which need rows 3-5) start sooner
                    nc.vector.tensor_copy(G_sb[:, 3, 384:C], g_accB[0][:])
                    nc.scalar.activation(G_sb[:, 4, 512:C], g_accB[1][:], COPY)
                    nc.scalar.activation(G_sb[:, 5, 640:C], g_accB[2][:], COPY)
                for n, (i, j) in enumerate(((4, 3), (5, 3), (5, 4))):
                    pt = ps_mm.tile([128, 128], BF16, tag="pmm", name=f"pt{i}{j}")
                    nc.tensor.transpose(
                        pt[:], G_sb[:, j, i * 128 : i * 128 + 128], ident[:]
                    )
                    if n % 2 == 0:
                        nc.vector.tensor_copy(
                            G_sb[:, i, j * 128 : j * 128 + 128], pt[:]
                        )
                    else:
                        nc.scalar.activation(
                            G_sb[:, i, j * 128 : j * 128 + 128], pt[:], COPY
                        )


                # ---- A = G @ W_k  (contraction over c) ----
                A_sb = apool.tile([128, CCH, C], BF16, tag="A")
                with tc.tile_pool(name="ps_A", bufs=2, space=PSUM) as ps_A:
                    for cp in range(CCH):
                        pA = ps_A.tile([128, C], F32, tag="pA")
                        for lo, hi in ((0, 512), (512, 768)):
                            for cch in range(CCH):
                                nc.tensor.matmul(
                                    pA[:, lo:hi],
                                    G_sb[:, cch, cp * 128 : (cp + 1) * 128],
                                    w_qkv[:, cch, C + lo : C + hi],
                                    start=(cch == 0),
                                    stop=(cch == CCH - 1),
                                    skip_group_check=True,
                                )
                        # split each evac at the psum bank boundary so DVE and
                        # ACT read different banks in parallel and the 2 psum
                        # bufs recycle fast enough to keep the A matmuls dense
                        nc.vector.tensor_copy(A_sb[:, cp, 0:512], pA[:, 0:512])
                        nc.scalar.activation(
                            A_sb[:, cp, 512:768], pA[:, 512:768], COPY
                        )
                        if cp == 2:
                            pop_q(1)  # cover the late-mirror/evac stalls
                    pop_q(2)  # fill the A-evac -> kv dependency bubble

                # ---- kv^T pairs = W_v_pair^T @ A_pair = (W_k^T G W_v)^T,
                # then gelu(kv^T * scale) into block-diag pair tiles ----
                # each pair's kv^T accumulates in its own (bank-aligned) psum
                # buffer so its gelu can read while the PE writes later pairs
                # in other banks (PE-W + ACT-R same bank is a HW error)
                with tc.tile_pool(name="ps_kv", bufs=4, space=PSUM) as ps_kv:
                    for pr in range(NPAIR):
                        kv_pr = ps_kv.tile([128, 512], F32, tag="kv", name="kv_pr")
                        for cch in range(CCH):
                            nc.tensor.matmul(
                                kv_pr[:, 0:128],
                                w_qkv[:, cch, 2 * C + pr * 128 : 2 * C + (pr + 1) * 128],
                                A_sb[:, cch, pr * 128 : (pr + 1) * 128],
                                start=(cch == 0),
                                stop=(cch == CCH - 1),
                                skip_group_check=True,
                            )
                        # gelu right after this pair's stop so ACT chews
                        # through the 12 small gelus while PE does later pairs
                        nc.scalar.activation(
                            gkv[0:64, pr, 0:64],
                            kv_pr[0:64, 0:64],
                            GELU,
                            scale=SCALE,
                        )
                        nc.scalar.activation(
                            gkv[64:128, pr, 64:128],
                            kv_pr[64:128, 64:128],
                            GELU,
                            scale=SCALE,
                        )

                    # ---- M = blockdiag_h(gelu(kv_h)) @ w_proj  (per pair):
                    # lhsT = gelu(kv^T) blockdiag so lhsT^T = gelu(kv) blockdiag
                    for pr in range(NPAIR):
                        pM0 = ps_mm.tile([128, 512], F32, tag="pmm")
                        pM1 = ps_mm.tile([128, 256], F32, tag="pmm")
                        nc.tensor.matmul(
                            pM0[:], gkv[:, pr, :], w_proj[:, pr, 0:512],
                            skip_group_check=True,
                        )
                        nc.tensor.matmul(
                            pM1[:], gkv[:, pr, :], w_proj[:, pr, 512:768],
                            skip_group_check=True,
                        )
                        # DVE-heavy evac split: ACT is busy with the gelus
                        nc.vector.tensor_copy(M_sb[:, pr, 0:512], pM0[:])
                        if pr % 2 == 0:
                            nc.vector.tensor_copy(M_sb[:, pr, 512:768], pM1[:])
                        else:
                            nc.scalar.activation(M_sb[:, pr, 512:768], pM1[:], COPY)
                    pop_q(2)  # fill the M-evac -> pass-2 dependency bubble

                # ============ pass 2: y = gelu(q) @ M + b  (single GEMM) ====
                with tc.tile_pool(name="ps_y", bufs=5, space=PSUM) as ps_y:
                  for ts in range(NTS):
                    for tc4 in range(4):
                        tsl = slice(
                            ts * 512 + tc4 * 128, ts * 512 + tc4 * 128 + 128
                        )
                        py0 = ps_y.tile([128, 512], F32, tag="py")
                        py1 = ps_y.tile([128, 256], F32, tag="py", name="py1")
                        for cch in range(CCH):
                            last = cch == CCH - 1
                            nc.tensor.matmul(
                                py0[:],
                                gqT[:, cch, tsl],
                                M_sb[:, cch, 0:512],
                                start=(cch == 0),
                                stop=last,
                                skip_group_check=True,
                            )
                            nc.tensor.matmul(
                                py1[:],
                                gqT[:, cch, tsl],
                                M_sb[:, cch, 512:768],
                                start=(cch == 0),
                                stop=last,
                                skip_group_check=True,
                            )
                        y_sb = ypool.tile([128, C], F32)
                        nc.vector.tensor_add(y_sb[:, 0:512], py0[:], b_bc[:, 0:512])
                        nc.vector.tensor_add(
                            y_sb[:, 512:768], py1[:], b_bc[:, 512:768]
                        )
                        t0 = ts * 512 + tc4 * 128
                        if b == BPC - 1 and ts == NTS - 1 and tc4 >= 2:
                            # drain the last tiles on both rings in halves so
                            # transfer+receipt latencies overlap at the tail
                            nc.sync.dma_start(
                                y_d[b, t0 : t0 + 128, 0:384], y_sb[:, 0:384]
                            )
                            nc.scalar.dma_start(
                                y_d[b, t0 : t0 + 128, 384:C], y_sb[:, 384:C]
                            )
                        else:
                            # alternate the two HWDGE rings (SP / ACT) so the
                            # output queue drains twice as fast
                            eng = nc.sync if tc4 % 2 == 0 else nc.scalar
                            eng.dma_start(y_d[b, t0 : t0 + 128, :], y_sb[:])

    nc.compile()
    return nc


_cached_nc = None


def kernel(x, w_qkv, w_proj, b_proj):
    global _cached_nc
    if _cached_nc is None:
        _cached_nc = _build_program()
    nc = _cached_nc

    x = np.ascontiguousarray(x, dtype=np.float32)
    in_maps = [
        {
            "x": x[i * BPC : (i + 1) * BPC],
            "w_qkv": np.asarray(w_qkv, dtype=np.float32),
            "w_proj": np.asarray(w_proj, dtype=np.float32),
            "b_proj": np.asarray(b_proj, dtype=np.float32),
        }
        for i in range(NCORES)
    ]
    last_err = None
    for _attempt in range(3):
        try:
            res = run_bass_kernel_spmd(nc, in_maps, core_ids=list(range(NCORES)))
            out = np.concatenate(
                [res.results[i]["y"] for i in range(NCORES)], axis=0
            )
            return out.astype(np.float32)
        except Exception as e:  # transient NRT device errors recover on retry
            last_err = e
    raise last_err



# revision 49
# speedup vs baseline: 1.0050x; 1.0033x over previous
"""Trainium2 Bass kernel for AttentionSimple (linear/kernelized attention).

Computes, for x:[B,N,C], w_qkv:[C,3C], w_proj:[C,C], b_proj:[C]:
    qkv = x @ w_qkv -> split q,k,v per head (H=12, D=64)
    kv  = (k^T v) * D^-0.5          per (b, h)     [D, D]
    out = gelu(q) @ gelu(kv)        per (b, h)     [N, D]
    y   = out @ w_proj + b_proj

Sharding: data-parallel over batch B=16 across 8 NeuronCores (2 batches/core).
All matmuls run in bf16 with fp32 PSUM accumulation.

Algorithm (FLOP-reduced):
  * kv goes through the Gram matrix G = x^T x (symmetric: only the upper
    block-triangle is computed; lower blocks are PE-transposed mirrors),
    then A = G @ W_k and kv^T_pair = W_v_pair^T @ A_pair -- swapping the
    Wk/Wv roles around symmetric G directly yields kv^T, which is the
    orientation the M-build wants as a stationary operand.
  * the per-head attention matmul is folded into the projection:
    y = gelu(q) @ M + b  with  M = blockdiag_h(gelu(kv_h)) @ w_proj,
    eliminating the whole attention pass and its PSUM evacuations.

Schedule per core (per batch b):
  boot:    zero-scratch HAM-warmup matmuls bridge the first x DMA wait.
  pass 1a: x slices stream (SWDGE fp32->bf16) into a persistent full-batch
           x_all tile; per 128-token chunk: PE transposes build x^T slices
           (batched strided DVE evacuation) and G rows 0-2 accumulate in
           PSUM; q^T chunks (lhsT = w_q chunk, rhs = x^T, gelu fused into
           the ACT evacuation) run two slices behind the transposes so the
           initial w_q DMA latency stays hidden.
  pass 1b: G rows 3-5 read the SBUF-resident x_all (no HBM reload); mirror
           transposes of finished G rows interleave on a dedicated psum
           bank; leftover q units drain here.
  A/kv/M:  A = G @ W_k (evacs split DVE/ACT at the bank boundary); per-pair
           kv^T accumulates in its own rotating psum bank with the gelu
           issued right after that pair's last matmul; M built per pair
           from the block-diagonal gelu(kv^T) tiles; remaining q units
           fill the dependency bubbles.
  pass 2:  y = gqT^T @ M + b as 512/256-wide matmuls into a 4-deep psum
           pool; bias added on DVE; output DMAs alternate the two HWDGE
           rings (SP/ACT); next batch's x prefetches underneath.

Self-contained: hardcodes shapes; builds the Bass program, runs it SPMD on
cores 0-7 via bass_utils.run_bass_kernel_spmd, returns the gathered output.
"""

import numpy as np

import concourse.bacc as bacc
import concourse.bass as bass
import concourse.mybir as mybir
import concourse.tile as tile
from concourse import masks
from concourse.bass_utils import run_bass_kernel_spmd

F32 = mybir.dt.float32
BF16 = mybir.dt.bfloat16
GELU = mybir.ActivationFunctionType.Gelu
COPY = mybir.ActivationFunctionType.Copy
PSUM = bass.MemorySpace.PSUM

B, N, C = 16, 4096, 768
H, D = 12, 64
SCALE = D**-0.5
NCORES = 8
BPC = B // NCORES  # batches per core
CCH = C // 128  # 6 column chunks of 128
NTS = N // 512  # 8 slices of 512 tokens
NPAIR = H // 2  # 6 head pairs (128 cols each)


def _build_program():
    nc = bacc.Bacc("TRN2", target_bir_lowering=False, debug=False)

    x_d = nc.dram_tensor("x", [BPC, N, C], F32, kind="ExternalInput").ap()
    wq_d = nc.dram_tensor("w_qkv", [C, 3 * C], F32, kind="ExternalInput").ap()
    wp_d = nc.dram_tensor("w_proj", [C, C], F32, kind="ExternalInput").ap()
    bp_d = nc.dram_tensor("b_proj", [C], F32, kind="ExternalInput").ap()
    y_d = nc.dram_tensor("y", [BPC, N, C], F32, kind="ExternalOutput").ap()

    with tile.TileContext(nc) as tc:
        with (
            tc.tile_pool(name="weights", bufs=1) as wpool,
            tc.tile_pool(name="acts", bufs=1) as apool,
            tc.tile_pool(name="xin", bufs=1) as xapool,
            tc.tile_pool(name="xt", bufs=4) as xtpool,
            tc.tile_pool(name="yout", bufs=5) as ypool,
            tc.tile_pool(name="ps_mm", bufs=3, space=PSUM) as ps_mm,
        ):
            # ---- boot order matters: gpsimd executes in FIFO order, so the
            # tiny scratch memset goes first (unblocks the PE warmup), then
            # the first x slice, then the identity, then the second slice.
            ident = wpool.tile([128, 128], BF16)
            scratch = wpool.tile([128, 128], BF16)
            nc.gpsimd.memset(scratch[:], 0.0)

            # ---- full per-batch x in bf16, [c on part? no: t-chunks, 768]:
            # x_all[p, tch, c] = x[b, tch*128+p, c].  Loaded once per batch
            # (no pass-1b reload); both G passes + transposes read it.
            x_all_tiles = {}

            def get_x_all(b):
                if b not in x_all_tiles:
                    x_all_tiles[b] = xapool.tile(
                        [128, N // 128, C], BF16, tag="x_all", name=f"x_all{b}"
                    )
                return x_all_tiles[b]

            # ---- prefetch the first token slice before the big weight DMAs
            # so the SWDGE rings deliver x(0,0) immediately and the PE can
            # start transposing while weights stream in.
            def load_x(b, ts):
                xa = get_x_all(b)
                for tc4 in range(4):
                    t0 = ts * 512 + tc4 * 128
                    nc.gpsimd.dma_start(
                        xa[:, ts * 4 + tc4, :], x_d[b, t0 : t0 + 128, :]
                    )

            load_x(0, 0)
            masks.make_identity(nc, ident[:])
            load_x(0, 1)  # second slice before weights: slot 1 needs it early

            # ---- HAM warmup: throwaway matmuls on the zero scratch bridge
            # the x(0,0) DMA wait and flip the PE clock gate to 8/8 before
            # real work starts
            with tc.tile_pool(name="ps_warm", bufs=1, space=PSUM) as ps_warm:
                warm = ps_warm.tile([128, 128], F32)
                for _ in range(40):
                    nc.tensor.matmul(warm[:], scratch[:], scratch[:])

            # ---- weights: q 512-col slices first (needed first), then k/v,
            # proj ----
            w_qkv = wpool.tile([128, CCH, 3 * C], BF16)  # 27.6KB/part
            w_proj = wpool.tile([128, CCH, C], BF16)  # 9.2KB/part
            for lo, hi in ((0, 512), (512, 768)):  # q slices (needed first)
                for cch in range(CCH):
                    nc.gpsimd.dma_start(
                        w_qkv[:, cch, lo:hi], wq_d[cch * 128 : (cch + 1) * 128, lo:hi]
                    )
            # Remaining weight loads are deferred into the first batch's
            # ts-loop (3 per slice) so their issue cost and ring bandwidth
            # don't delay the x-tile prefetches. Deps still order correctly.
            b_bc = wpool.tile([128, C], F32)
            deferred_w = []
            for cch in range(CCH):  # k part (A-stage), then v part (kv-stage)
                deferred_w.append(
                    (
                        w_qkv[:, cch, C : 2 * C],
                        wq_d[cch * 128 : (cch + 1) * 128, C : 2 * C],
                    )
                )
            for cch in range(CCH):
                deferred_w.append(
                    (w_qkv[:, cch, 2 * C :], wq_d[cch * 128 : (cch + 1) * 128, 2 * C :])
                )
            for cch in range(CCH):
                deferred_w.append(
                    (w_proj[:, cch, :], wp_d[cch * 128 : (cch + 1) * 128, :])
                )
            deferred_w.append((b_bc[:], bp_d.unsqueeze(0).partition_broadcast(128)))
            deferred_w.reverse()

            # gkv: per-pair block-diagonal [128,128] with gelu(kv^T*scale) of
            # the even head at [0:64,0:64] and odd head at [64:,64:].  One
            # buffer for all batches: the off-diagonal zeros are written once
            # here and never touched again (gelus only write the diagonals).
            gkv = apool.tile([128, NPAIR, 128], BF16, tag="gkv")
            nc.gpsimd.memset(gkv[:], 0.0)

            for b in range(BPC):
                # gqT: q^T with gelu applied, [c=768, t=4096] as 6 chunks
                gqT = apool.tile([128, CCH, N], BF16, tag="gqT")
                # M: folded projection, rows pair-chunked: M = blkdiag(gkv)@Wp
                M_sb = apool.tile([128, NPAIR, C], BF16, tag="M")
                # Gram matrix G = x^T x (bf16), used for kv = W_k^T G W_v
                G_sb = apool.tile([128, CCH, C], BF16, tag="G")

                x_all = get_x_all(b)

                # q-chunk units (ts, jch) are emitted 2 token-slices behind
                # the transposes that produce their xT input: q(ts) runs in
                # slot ts+2.  This hides the initial w_q DMA latency behind
                # two slots of transpose+G work; the ~12 units left over at
                # the end of pass 1a are drained in pass 1b and in the
                # A/kv/M dependency bubbles.
                xT_slices = {}

                def q_unit(ts, jch):
                    xTs = xT_slices[ts]
                    pq = ps_mm.tile([128, 512], F32, tag="pmm", name="pq")
                    for cch in range(CCH):
                        nc.tensor.matmul(
                            pq[:],
                            w_qkv[:, cch, jch * 128 : (jch + 1) * 128],
                            xTs[:, cch, 0:512],
                            start=(cch == 0),
                            stop=(cch == CCH - 1),
                        )
                    nc.scalar.activation(
                        gqT[:, jch, ts * 512 : ts * 512 + 512], pq[:], GELU
                    )

                q_iter = iter(
                    [(ts, jch) for ts in range(NTS) for jch in range(CCH)]
                )

                def pop_q(n):
                    for _ in range(n):
                        tj = next(q_iter, None)
                        if tj is None:
                            return
                        q_unit(*tj)

                # ===== pass 1a: q^T + upper-triangular G rows 0..2 ==========
                with tc.tile_pool(name="ps_gA", bufs=1, space=PSUM) as ps_gA:
                    g_acc = [
                        ps_gA.tile([128, C - ci * 128], F32, tag=f"g{ci}", name=f"gA{ci}")
                        for ci in range(3)
                    ]
                    for ts in range(NTS):
                        xT = xtpool.tile([128, CCH, 512], BF16)
                        xT_slices[ts] = xT
                        # slices 0,1 of each batch are prefetched earlier, so
                        # slot ts tops up slice ts+2
                        if ts + 2 <= NTS - 1:
                            load_x(b, ts + 2)
                        elif b + 1 < BPC:
                            load_x(b + 1, ts + 2 - NTS)
                        for _ in range(3):
                            if deferred_w:
                                dst, srcap = deferred_w.pop()
                                nc.gpsimd.dma_start(dst, srcap)
                        # q of slice ts-2 goes FIRST: its inputs (xT, w_q) are
                        # ready, so it must not sit behind this slot's
                        # transposes in the in-order PE queue when the x DMA
                        # is late (startup).
                        if ts >= 2:
                            pop_q(CCH)
                        for tc4 in range(4):
                            x_bf = x_all[:, ts * 4 + tc4, :]
                            tr = ps_mm.tile([128, CCH * 128], BF16, tag="pmm")
                            for cch in range(CCH):
                                nc.tensor.transpose(
                                    tr[:, cch * 128 : (cch + 1) * 128],
                                    x_bf[:, cch * 128 : (cch + 1) * 128],
                                    ident[:],
                                )
                            nc.vector.tensor_copy(
                                xT[:, :, tc4 * 128 : tc4 * 128 + 128],
                                tr[:].rearrange("p (c f) -> p c f", c=CCH),
                            )
                            # G rows ci, cols [ci*128:768), accumulated over
                            # all 32 token chunks; each 512/256-col split owns
                            # its psum bank so start=(first chunk) is safe.
                            first = ts == 0 and tc4 == 0
                            last = ts == NTS - 1 and tc4 == 3
                            for ci in range(3):
                                w = C - ci * 128
                                for lo in range(0, w, 512):
                                    hi = min(lo + 512, w)
                                    nc.tensor.matmul(
                                        g_acc[ci][:, lo:hi],
                                        x_bf[:, ci * 128 : (ci + 1) * 128],
                                        x_bf[:, ci * 128 + lo : ci * 128 + hi],
                                        start=first,
                                        stop=last,
                                        skip_group_check=True,
                                    )
                    # split the tail evacs across DVE and ACT: the pass-1b
                    # mirrors read these rows and stall if DVE does all three
                    nc.vector.tensor_copy(G_sb[:, 0, 0:C], g_acc[0][:])
                    nc.scalar.activation(G_sb[:, 1, 128:C], g_acc[1][:], COPY)
                    nc.vector.tensor_copy(G_sb[:, 2, 256:C], g_acc[2][:])

                # ===== pass 1b: G rows 3..5 (x cols 384: from SBUF) =========
                early_mirrors = [
                    (1, 0), (2, 0), (2, 1), (3, 0), (3, 1), (3, 2),
                    (4, 0), (4, 1), (4, 2), (5, 0), (5, 1), (5, 2),
                ][::-1]
                with (
                    tc.tile_pool(name="ps_gB", bufs=1, space=PSUM) as ps_gB,
                    tc.tile_pool(name="ps_pt", bufs=1, space=PSUM) as ps_pt,
                ):
                    g_accB = [
                        ps_gB.tile([128, C - ci * 128], F32, tag=f"g{ci}", name=f"gB{ci}")
                        for ci in range(3, CCH)
                    ]

                    for ts in range(NTS):
                        if ts > 0:
                            pop_q(1)
                        # fill pass 1b with mirror transposes whose sources
                        # (G rows 0-2) were finished in pass 1a; they use a
                        # dedicated psum bank so they don't contend with the
                        # q units' ps_mm rotation
                        for _ in range(2):
                            if early_mirrors:
                                i, j = early_mirrors.pop()
                                pt = ps_pt.tile(
                                    [128, 128], BF16, tag="pt", name=f"pt{i}{j}"
                                )
                                nc.tensor.transpose(
                                    pt[:],
                                    G_sb[:, j, i * 128 : i * 128 + 128],
                                    ident[:],
                                )
                                nc.vector.tensor_copy(
                                    G_sb[:, i, j * 128 : j * 128 + 128], pt[:]
                                )
                        for tc4 in range(4):
                            x_hf = x_all[:, ts * 4 + tc4, 384:C]
                            first = ts == 0 and tc4 == 0
                            last = ts == NTS - 1 and tc4 == 3
                            for k, ci in enumerate(range(3, CCH)):
                                off = ci * 128 - 384
                                nc.tensor.matmul(
                                    g_accB[k][:],
                                    x_hf[:, off : off + 128],
                                    x_hf[:, off:],
                                    start=first,
                                    stop=last,
                                    skip_group_check=True,
                                )
                    # parallelize the tail evacs across DVE and ACT so the
                    # late mirrors (which need rows 3-5) start sooner
                    nc.vector.tensor_copy(G_sb[:, 3, 384:C], g_accB[0][:])
                    nc.scalar.activation(G_sb[:, 4, 512:C], g_accB[1][:], COPY)
                    nc.scalar.activation(G_sb[:, 5, 640:C], g_accB[2][:], COPY)
                for n, (i, j) in enumerate(((4, 3), (5, 3), (5, 4))):
                    pt = ps_mm.tile([128, 128], BF16, tag="pmm", name=f"pt{i}{j}")
                    nc.tensor.transpose(
                        pt[:], G_sb[:, j, i * 128 : i * 128 + 128], ident[:]
                    )
                    if n % 2 == 0:
                        nc.vector.tensor_copy(
                            G_sb[:, i, j * 128 : j * 128 + 128], pt[:]
                        )
                    else:
                        nc.scalar.activation(
                            G_sb[:, i, j * 128 : j * 128 + 128], pt[:], COPY
                        )


                # ---- A = G @ W_k  (contraction over c) ----
                A_sb = apool.tile([128, CCH, C], BF16, tag="A")
                with tc.tile_pool(name="ps_A", bufs=2, space=PSUM) as ps_A:
                    for cp in range(CCH):
                        # separate single-bank tiles per half: each half's
                        # buffer recycles as soon as its own evac finishes
                        pA0 = ps_A.tile([128, 512], F32, tag="pA0")
                        pA1 = ps_A.tile([128, 256], F32, tag="pA1")
                        for lo, hi, pA in ((0, 512, pA0), (512, 768, pA1)):
                            for cch in range(CCH):
                                nc.tensor.matmul(
                                    pA[:, 0 : hi - lo],
                                    G_sb[:, cch, cp * 128 : (cp + 1) * 128],
                                    w_qkv[:, cch, C + lo : C + hi],
                                    start=(cch == 0),
                                    stop=(cch == CCH - 1),
                                    skip_group_check=True,
                                )
                        # DVE and ACT evacuate different banks in parallel
                        nc.vector.tensor_copy(A_sb[:, cp, 0:512], pA0[:])
                        nc.scalar.activation(
                            A_sb[:, cp, 512:768], pA1[:], COPY
                        )
                        if cp == 2:
                            pop_q(1)  # cover the late-mirror/evac stalls
                    pop_q(2)  # fill the A-evac -> kv dependency bubble

                # ---- kv^T pairs = W_v_pair^T @ A_pair = (W_k^T G W_v)^T,
                # then gelu(kv^T * scale) into block-diag pair tiles ----
                # each pair's kv^T accumulates in its own (bank-aligned) psum
                # buffer so its gelu can read while the PE writes later pairs
                # in other banks (PE-W + ACT-R same bank is a HW error)
                with tc.tile_pool(name="ps_kv", bufs=4, space=PSUM) as ps_kv:
                    for pr in range(NPAIR):
                        kv_pr = ps_kv.tile([128, 512], F32, tag="kv", name="kv_pr")
                        for cch in range(CCH):
                            nc.tensor.matmul(
                                kv_pr[:, 0:128],
                                w_qkv[:, cch, 2 * C + pr * 128 : 2 * C + (pr + 1) * 128],
                                A_sb[:, cch, pr * 128 : (pr + 1) * 128],
                                start=(cch == 0),
                                stop=(cch == CCH - 1),
                                skip_group_check=True,
                            )
                        # gelu right after this pair's stop so ACT chews
                        # through the 12 small gelus while PE does later pairs
                        nc.scalar.activation(
                            gkv[0:64, pr, 0:64],
                            kv_pr[0:64, 0:64],
                            GELU,
                            scale=SCALE,
                        )
                        nc.scalar.activation(
                            gkv[64:128, pr, 64:128],
                            kv_pr[64:128, 64:128],
                            GELU,
                            scale=SCALE,
                        )

                    # ---- M = blockdiag_h(gelu(kv_h)) @ w_proj  (per pair):
                    # lhsT = gelu(kv^T) blockdiag so lhsT^T = gelu(kv) blockdiag
                    for pr in range(NPAIR):
                        pM0 = ps_mm.tile([128, 512], F32, tag="pmm")
                        pM1 = ps_mm.tile([128, 256], F32, tag="pmm")
                        nc.tensor.matmul(
                            pM0[:], gkv[:, pr, :], w_proj[:, pr, 0:512],
                            skip_group_check=True,
                        )
                        nc.tensor.matmul(
                            pM1[:], gkv[:, pr, :], w_proj[:, pr, 512:768],
                            skip_group_check=True,
                        )
                        # DVE-heavy evac split: ACT is busy with the gelus
                        nc.vector.tensor_copy(M_sb[:, pr, 0:512], pM0[:])
                        if pr % 2 == 0:
                            nc.vector.tensor_copy(M_sb[:, pr, 512:768], pM1[:])
                        else:
                            nc.scalar.activation(M_sb[:, pr, 512:768], pM1[:], COPY)
                    pop_q(2)  # fill the M-evac -> pass-2 dependency bubble

                # ============ pass 2: y = gelu(q) @ M + b  (single GEMM) ====
                with tc.tile_pool(name="ps_y", bufs=5, space=PSUM) as ps_y:
                  for ts in range(NTS):
                    for tc4 in range(4):
                        tsl = slice(
                            ts * 512 + tc4 * 128, ts * 512 + tc4 * 128 + 128
                        )
                        py0 = ps_y.tile([128, 512], F32, tag="py")
                        py1 = ps_y.tile([128, 256], F32, tag="py", name="py1")
                        for cch in range(CCH):
                            last = cch == CCH - 1
                            nc.tensor.matmul(
                                py0[:],
                                gqT[:, cch, tsl],
                                M_sb[:, cch, 0:512],
                                start=(cch == 0),
                                stop=last,
                                skip_group_check=True,
                            )
                            nc.tensor.matmul(
                                py1[:],
                                gqT[:, cch, tsl],
                                M_sb[:, cch, 512:768],
                                start=(cch == 0),
                                stop=last,
                                skip_group_check=True,
                            )
                        y_sb = ypool.tile([128, C], F32)
                        nc.vector.tensor_add(y_sb[:, 0:512], py0[:], b_bc[:, 0:512])
                        nc.vector.tensor_add(
                            y_sb[:, 512:768], py1[:], b_bc[:, 512:768]
                        )
                        t0 = ts * 512 + tc4 * 128
                        if b == BPC - 1 and ts == NTS - 1 and tc4 >= 2:
                            # drain the last tiles on both rings in halves so
                            # transfer+receipt latencies overlap at the tail
                            nc.sync.dma_start(
                                y_d[b, t0 : t0 + 128, 0:384], y_sb[:, 0:384]
                            )
                            nc.scalar.dma_start(
                                y_d[b, t0 : t0 + 128, 384:C], y_sb[:, 384:C]
                            )
                        else:
                            # alternate the two HWDGE rings (SP / ACT) so the
                            # output queue drains twice as fast
                            eng = nc.sync if tc4 % 2 == 0 else nc.scalar
                            eng.dma_start(y_d[b, t0 : t0 + 128, :], y_sb[:])

    nc.compile()
    return nc


_cached_nc = None


def kernel(x, w_qkv, w_proj, b_proj):
    global _cached_nc
    if _cached_nc is None:
        _cached_nc = _build_program()
    nc = _cached_nc

    x = np.ascontiguousarray(x, dtype=np.float32)
    in_maps = [
        {
            "x": x[i * BPC : (i + 1) * BPC],
            "w_qkv": np.asarray(w_qkv, dtype=np.float32),
            "w_proj": np.asarray(w_proj, dtype=np.float32),
            "b_proj": np.asarray(b_proj, dtype=np.float32),
        }
        for i in range(NCORES)
    ]
    last_err = None
    for _attempt in range(3):
        try:
            res = run_bass_kernel_spmd(nc, in_maps, core_ids=list(range(NCORES)))
            out = np.concatenate(
                [res.results[i]["y"] for i in range(NCORES)], axis=0
            )
            return out.astype(np.float32)
        except Exception as e:  # transient NRT device errors recover on retry
            last_err = e
    raise last_err



# revision 51
# speedup vs baseline: 1.0068x; 1.0019x over previous
"""Trainium2 Bass kernel for AttentionSimple (linear/kernelized attention).

Computes, for x:[B,N,C], w_qkv:[C,3C], w_proj:[C,C], b_proj:[C]:
    qkv = x @ w_qkv -> split q,k,v per head (H=12, D=64)
    kv  = (k^T v) * D^-0.5          per (b, h)     [D, D]
    out = gelu(q) @ gelu(kv)        per (b, h)     [N, D]
    y   = out @ w_proj + b_proj

Sharding: data-parallel over batch B=16 across 8 NeuronCores (2 batches/core).
All matmuls run in bf16 with fp32 PSUM accumulation.

Algorithm (FLOP-reduced):
  * kv goes through the Gram matrix G = x^T x (symmetric: only the upper
    block-triangle is computed; lower blocks are PE-transposed mirrors),
    then A = G @ W_k and kv^T_pair = W_v_pair^T @ A_pair -- swapping the
    Wk/Wv roles around symmetric G directly yields kv^T, which is the
    orientation the M-build wants as a stationary operand.
  * the per-head attention matmul is folded into the projection:
    y = gelu(q) @ M + b  with  M = blockdiag_h(gelu(kv_h)) @ w_proj,
    eliminating the whole attention pass and its PSUM evacuations.

Schedule per core (per batch b):
  boot:    zero-scratch HAM-warmup matmuls bridge the first x DMA wait.
  pass 1a: x slices stream (SWDGE fp32->bf16) into a persistent full-batch
           x_all tile; per 128-token chunk: PE transposes build x^T slices
           (batched strided DVE evacuation) and G rows 0-2 accumulate in
           PSUM; q^T chunks (lhsT = w_q chunk, rhs = x^T, gelu fused into
           the ACT evacuation) run two slices behind the transposes so the
           initial w_q DMA latency stays hidden.
  pass 1b: G rows 3-5 read the SBUF-resident x_all (no HBM reload); mirror
           transposes of finished G rows interleave on a dedicated psum
           bank; leftover q units drain here.
  A/kv/M:  A = G @ W_k (evacs split DVE/ACT at the bank boundary); per-pair
           kv^T accumulates in its own rotating psum bank with the gelu
           issued right after that pair's last matmul; M built per pair
           from the block-diagonal gelu(kv^T) tiles; remaining q units
           fill the dependency bubbles.
  pass 2:  y = gqT^T @ M + b as 512/256-wide matmuls into a 4-deep psum
           pool; bias added on DVE; output DMAs alternate the two HWDGE
           rings (SP/ACT); next batch's x prefetches underneath.

Self-contained: hardcodes shapes; builds the Bass program, runs it SPMD on
cores 0-7 via bass_utils.run_bass_kernel_spmd, returns the gathered output.
"""

import numpy as np

import concourse.bacc as bacc
import concourse.bass as bass
import concourse.mybir as mybir
import concourse.tile as tile
from concourse import masks
from concourse.bass_utils import run_bass_kernel_spmd

F32 = mybir.dt.float32
BF16 = mybir.dt.bfloat16
GELU = mybir.ActivationFunctionType.Gelu
COPY = mybir.ActivationFunctionType.Copy
PSUM = bass.MemorySpace.PSUM

B, N, C = 16, 4096, 768
H, D = 12, 64
SCALE = D**-0.5
NCORES = 8
BPC = B // NCORES  # batches per core
CCH = C // 128  # 6 column chunks of 128
NTS = N // 512  # 8 slices of 512 tokens
NPAIR = H // 2  # 6 head pairs (128 cols each)


def _build_program():
    nc = bacc.Bacc("TRN2", target_bir_lowering=False, debug=False)

    x_d = nc.dram_tensor("x", [BPC, N, C], F32, kind="ExternalInput").ap()
    wq_d = nc.dram_tensor("w_qkv", [C, 3 * C], F32, kind="ExternalInput").ap()
    wp_d = nc.dram_tensor("w_proj", [C, C], F32, kind="ExternalInput").ap()
    bp_d = nc.dram_tensor("b_proj", [C], F32, kind="ExternalInput").ap()
    y_d = nc.dram_tensor("y", [BPC, N, C], F32, kind="ExternalOutput").ap()

    with tile.TileContext(nc) as tc:
        with (
            tc.tile_pool(name="weights", bufs=1) as wpool,
            tc.tile_pool(name="acts", bufs=1) as apool,
            tc.tile_pool(name="xin", bufs=1) as xapool,
            tc.tile_pool(name="xt", bufs=4) as xtpool,
            tc.tile_pool(name="yout", bufs=5) as ypool,
            tc.tile_pool(name="ps_mm", bufs=3, space=PSUM) as ps_mm,
        ):
            # ---- boot order matters: gpsimd executes in FIFO order, so the
            # tiny scratch memset goes first (unblocks the PE warmup), then
            # the first x slice, then the identity, then the second slice.
            ident = wpool.tile([128, 128], BF16)
            scratch = wpool.tile([128, 128], BF16)
            nc.gpsimd.memset(scratch[:], 0.0)

            # ---- full per-batch x in bf16, [c on part? no: t-chunks, 768]:
            # x_all[p, tch, c] = x[b, tch*128+p, c].  Loaded once per batch
            # (no pass-1b reload); both G passes + transposes read it.
            x_all_tiles = {}

            def get_x_all(b):
                if b not in x_all_tiles:
                    x_all_tiles[b] = xapool.tile(
                        [128, N // 128, C], BF16, tag="x_all", name=f"x_all{b}"
                    )
                return x_all_tiles[b]

            # ---- prefetch the first token slice before the big weight DMAs
            # so the SWDGE rings deliver x(0,0) immediately and the PE can
            # start transposing while weights stream in.
            def load_x(b, ts):
                xa = get_x_all(b)
                for tc4 in range(4):
                    t0 = ts * 512 + tc4 * 128
                    nc.gpsimd.dma_start(
                        xa[:, ts * 4 + tc4, :], x_d[b, t0 : t0 + 128, :]
                    )

            load_x(0, 0)
            masks.make_identity(nc, ident[:])
            load_x(0, 1)  # second slice before weights: slot 1 needs it early

            # ---- HAM warmup: throwaway matmuls on the zero scratch bridge
            # the x(0,0) DMA wait and flip the PE clock gate to 8/8 before
            # real work starts
            with tc.tile_pool(name="ps_warm", bufs=1, space=PSUM) as ps_warm:
                warm = ps_warm.tile([128, 128], F32)
                for _ in range(40):
                    nc.tensor.matmul(warm[:], scratch[:], scratch[:])

            # ---- weights: q 512-col slices first (needed first), then k/v,
            # proj ----
            w_qkv = wpool.tile([128, CCH, 3 * C], BF16)  # 27.6KB/part
            w_proj = wpool.tile([128, CCH, C], BF16)  # 9.2KB/part
            for lo, hi in ((0, 512), (512, 768)):  # q slices (needed first)
                for cch in range(CCH):
                    nc.gpsimd.dma_start(
                        w_qkv[:, cch, lo:hi], wq_d[cch * 128 : (cch + 1) * 128, lo:hi]
                    )
            # Remaining weight loads are deferred into the first batch's
            # ts-loop (3 per slice) so their issue cost and ring bandwidth
            # don't delay the x-tile prefetches. Deps still order correctly.
            b_bc = wpool.tile([128, C], F32)
            deferred_w = []
            for cch in range(CCH):  # k part (A-stage), then v part (kv-stage)
                deferred_w.append(
                    (
                        w_qkv[:, cch, C : 2 * C],
                        wq_d[cch * 128 : (cch + 1) * 128, C : 2 * C],
                    )
                )
            for cch in range(CCH):
                deferred_w.append(
                    (w_qkv[:, cch, 2 * C :], wq_d[cch * 128 : (cch + 1) * 128, 2 * C :])
                )
            for cch in range(CCH):
                deferred_w.append(
                    (w_proj[:, cch, :], wp_d[cch * 128 : (cch + 1) * 128, :])
                )
            deferred_w.append((b_bc[:], bp_d.unsqueeze(0).partition_broadcast(128)))
            deferred_w.reverse()

            # gkv: per-pair block-diagonal [128,128] with gelu(kv^T*scale) of
            # the even head at [0:64,0:64] and odd head at [64:,64:].  One
            # buffer for all batches: the off-diagonal zeros are written once
            # here and never touched again (gelus only write the diagonals).
            gkv = apool.tile([128, NPAIR, 128], BF16, tag="gkv")
            nc.gpsimd.memset(gkv[:], 0.0)

            for b in range(BPC):
                # gqT: q^T with gelu applied, [c=768, t=4096] as 6 chunks
                gqT = apool.tile([128, CCH, N], BF16, tag="gqT")
                # M: folded projection, rows pair-chunked: M = blkdiag(gkv)@Wp
                M_sb = apool.tile([128, NPAIR, C], BF16, tag="M")
                # Gram matrix G = x^T x (bf16), used for kv = W_k^T G W_v
                G_sb = apool.tile([128, CCH, C], BF16, tag="G")

                x_all = get_x_all(b)

                # q-chunk units (ts, jch) are emitted 2 token-slices behind
                # the transposes that produce their xT input: q(ts) runs in
                # slot ts+2.  This hides the initial w_q DMA latency behind
                # two slots of transpose+G work; the ~12 units left over at
                # the end of pass 1a are drained in pass 1b and in the
                # A/kv/M dependency bubbles.
                xT_slices = {}

                def q_unit(ts, jch):
                    xTs = xT_slices[ts]
                    pq = ps_mm.tile([128, 512], F32, tag="pmm", name="pq")
                    for cch in range(CCH):
                        nc.tensor.matmul(
                            pq[:],
                            w_qkv[:, cch, jch * 128 : (jch + 1) * 128],
                            xTs[:, cch, 0:512],
                            start=(cch == 0),
                            stop=(cch == CCH - 1),
                        )
                    nc.scalar.activation(
                        gqT[:, jch, ts * 512 : ts * 512 + 512], pq[:], GELU
                    )

                q_iter = iter(
                    [(ts, jch) for ts in range(NTS) for jch in range(CCH)]
                )

                def pop_q(n):
                    for _ in range(n):
                        tj = next(q_iter, None)
                        if tj is None:
                            return
                        q_unit(*tj)

                # ===== pass 1a: q^T + upper-triangular G rows 0..2 ==========
                with tc.tile_pool(name="ps_gA", bufs=1, space=PSUM) as ps_gA:
                    g_acc = [
                        ps_gA.tile([128, C - ci * 128], F32, tag=f"g{ci}", name=f"gA{ci}")
                        for ci in range(3)
                    ]
                    for ts in range(NTS):
                        xT = xtpool.tile([128, CCH, 512], BF16)
                        xT_slices[ts] = xT
                        # slices 0,1 of each batch are prefetched earlier, so
                        # slot ts tops up slice ts+2
                        if ts + 2 <= NTS - 1:
                            load_x(b, ts + 2)
                        elif b + 1 < BPC:
                            load_x(b + 1, ts + 2 - NTS)
                        # 2 deferred weight loads per slot keep the x-slice
                        # DMAs ahead of the transposes; the rest drain in 1b
                        for _ in range(2):
                            if deferred_w:
                                dst, srcap = deferred_w.pop()
                                nc.gpsimd.dma_start(dst, srcap)
                        # q of slice ts-2 goes FIRST: its inputs (xT, w_q) are
                        # ready, so it must not sit behind this slot's
                        # transposes in the in-order PE queue when the x DMA
                        # is late (startup).
                        if ts >= 2:
                            pop_q(CCH)
                        for tc4 in range(4):
                            x_bf = x_all[:, ts * 4 + tc4, :]
                            tr = ps_mm.tile([128, CCH * 128], BF16, tag="pmm")
                            for cch in range(CCH):
                                nc.tensor.transpose(
                                    tr[:, cch * 128 : (cch + 1) * 128],
                                    x_bf[:, cch * 128 : (cch + 1) * 128],
                                    ident[:],
                                )
                            nc.vector.tensor_copy(
                                xT[:, :, tc4 * 128 : tc4 * 128 + 128],
                                tr[:].rearrange("p (c f) -> p c f", c=CCH),
                            )
                            # G rows ci, cols [ci*128:768), accumulated over
                            # all 32 token chunks; each 512/256-col split owns
                            # its psum bank so start=(first chunk) is safe.
                            first = ts == 0 and tc4 == 0
                            last = ts == NTS - 1 and tc4 == 3
                            for ci in range(3):
                                w = C - ci * 128
                                for lo in range(0, w, 512):
                                    hi = min(lo + 512, w)
                                    nc.tensor.matmul(
                                        g_acc[ci][:, lo:hi],
                                        x_bf[:, ci * 128 : (ci + 1) * 128],
                                        x_bf[:, ci * 128 + lo : ci * 128 + hi],
                                        start=first,
                                        stop=last,
                                        skip_group_check=True,
                                    )
                    # split the tail evacs across DVE and ACT: the pass-1b
                    # mirrors read these rows and stall if DVE does all three
                    nc.vector.tensor_copy(G_sb[:, 0, 0:C], g_acc[0][:])
                    nc.scalar.activation(G_sb[:, 1, 128:C], g_acc[1][:], COPY)
                    nc.vector.tensor_copy(G_sb[:, 2, 256:C], g_acc[2][:])

                # ===== pass 1b: G rows 3..5 (x cols 384: from SBUF) =========
                early_mirrors = [
                    (1, 0), (2, 0), (2, 1), (3, 0), (3, 1), (3, 2),
                    (4, 0), (4, 1), (4, 2), (5, 0), (5, 1), (5, 2),
                ][::-1]
                with (
                    tc.tile_pool(name="ps_gB", bufs=1, space=PSUM) as ps_gB,
                    tc.tile_pool(name="ps_pt", bufs=1, space=PSUM) as ps_pt,
                ):
                    g_accB = [
                        ps_gB.tile([128, C - ci * 128], F32, tag=f"g{ci}", name=f"gB{ci}")
                        for ci in range(3, CCH)
                    ]

                    for ts in range(NTS):
                        if deferred_w:
                            dst, srcap = deferred_w.pop()
                            nc.gpsimd.dma_start(dst, srcap)
                        if ts > 0:
                            pop_q(1)
                        # fill pass 1b with mirror transposes whose sources
                        # (G rows 0-2) were finished in pass 1a; they use a
                        # dedicated psum bank so they don't contend with the
                        # q units' ps_mm rotation
                        for _ in range(2):
                            if early_mirrors:
                                i, j = early_mirrors.pop()
                                pt = ps_pt.tile(
                                    [128, 128], BF16, tag="pt", name=f"pt{i}{j}"
                                )
                                nc.tensor.transpose(
                                    pt[:],
                                    G_sb[:, j, i * 128 : i * 128 + 128],
                                    ident[:],
                                )
                                nc.vector.tensor_copy(
                                    G_sb[:, i, j * 128 : j * 128 + 128], pt[:]
                                )
                        for tc4 in range(4):
                            x_hf = x_all[:, ts * 4 + tc4, 384:C]
                            first = ts == 0 and tc4 == 0
                            last = ts == NTS - 1 and tc4 == 3
                            for k, ci in enumerate(range(3, CCH)):
                                off = ci * 128 - 384
                                nc.tensor.matmul(
                                    g_accB[k][:],
                                    x_hf[:, off : off + 128],
                                    x_hf[:, off:],
                                    start=first,
                                    stop=last,
                                    skip_group_check=True,
                                )
                    # parallelize the tail evacs across DVE and ACT so the
                    # late mirrors (which need rows 3-5) start sooner
                    nc.vector.tensor_copy(G_sb[:, 3, 384:C], g_accB[0][:])
                    nc.scalar.activation(G_sb[:, 4, 512:C], g_accB[1][:], COPY)
                    nc.scalar.activation(G_sb[:, 5, 640:C], g_accB[2][:], COPY)
                for n, (i, j) in enumerate(((4, 3), (5, 3), (5, 4))):
                    pt = ps_mm.tile([128, 128], BF16, tag="pmm", name=f"pt{i}{j}")
                    nc.tensor.transpose(
                        pt[:], G_sb[:, j, i * 128 : i * 128 + 128], ident[:]
                    )
                    if n % 2 == 0:
                        nc.vector.tensor_copy(
                            G_sb[:, i, j * 128 : j * 128 + 128], pt[:]
                        )
                    else:
                        nc.scalar.activation(
                            G_sb[:, i, j * 128 : j * 128 + 128], pt[:], COPY
                        )


                # ---- A = G @ W_k  (contraction over c) ----
                A_sb = apool.tile([128, CCH, C], BF16, tag="A")
                with tc.tile_pool(name="ps_A", bufs=2, space=PSUM) as ps_A:
                    for cp in range(CCH):
                        # separate single-bank tiles per half: each half's
                        # buffer recycles as soon as its own evac finishes
                        pA0 = ps_A.tile([128, 512], F32, tag="pA0")
                        pA1 = ps_A.tile([128, 256], F32, tag="pA1")
                        for lo, hi, pA in ((0, 512, pA0), (512, 768, pA1)):
                            for cch in range(CCH):
                                nc.tensor.matmul(
                                    pA[:, 0 : hi - lo],
                                    G_sb[:, cch, cp * 128 : (cp + 1) * 128],
                                    w_qkv[:, cch, C + lo : C + hi],
                                    start=(cch == 0),
                                    stop=(cch == CCH - 1),
                                    skip_group_check=True,
                                )
                        # DVE and ACT evacuate different banks in parallel
                        nc.vector.tensor_copy(A_sb[:, cp, 0:512], pA0[:])
                        nc.scalar.activation(
                            A_sb[:, cp, 512:768], pA1[:], COPY
                        )
                        if cp == 2:
                            pop_q(1)  # cover the late-mirror/evac stalls
                    pop_q(2)  # fill the A-evac -> kv dependency bubble

                # ---- kv^T pairs = W_v_pair^T @ A_pair = (W_k^T G W_v)^T,
                # then gelu(kv^T * scale) into block-diag pair tiles ----
                # each pair's kv^T accumulates in its own (bank-aligned) psum
                # buffer so its gelu can read while the PE writes later pairs
                # in other banks (PE-W + ACT-R same bank is a HW error)
                with tc.tile_pool(name="ps_kv", bufs=4, space=PSUM) as ps_kv:
                    for pr in range(NPAIR):
                        kv_pr = ps_kv.tile([128, 512], F32, tag="kv", name="kv_pr")
                        for cch in range(CCH):
                            nc.tensor.matmul(
                                kv_pr[:, 0:128],
                                w_qkv[:, cch, 2 * C + pr * 128 : 2 * C + (pr + 1) * 128],
                                A_sb[:, cch, pr * 128 : (pr + 1) * 128],
                                start=(cch == 0),
                                stop=(cch == CCH - 1),
                                skip_group_check=True,
                            )
                        # gelu right after this pair's stop so ACT chews
                        # through the 12 small gelus while PE does later pairs
                        nc.scalar.activation(
                            gkv[0:64, pr, 0:64],
                            kv_pr[0:64, 0:64],
                            GELU,
                            scale=SCALE,
                        )
                        nc.scalar.activation(
                            gkv[64:128, pr, 64:128],
                            kv_pr[64:128, 64:128],
                            GELU,
                            scale=SCALE,
                        )

                    # ---- M = blockdiag_h(gelu(kv_h)) @ w_proj  (per pair):
                    # lhsT = gelu(kv^T) blockdiag so lhsT^T = gelu(kv) blockdiag
                    for pr in range(NPAIR):
                        pM0 = ps_mm.tile([128, 512], F32, tag="pmm")
                        pM1 = ps_mm.tile([128, 256], F32, tag="pmm")
                        nc.tensor.matmul(
                            pM0[:], gkv[:, pr, :], w_proj[:, pr, 0:512],
                            skip_group_check=True,
                        )
                        nc.tensor.matmul(
                            pM1[:], gkv[:, pr, :], w_proj[:, pr, 512:768],
                            skip_group_check=True,
                        )
                        # DVE-heavy evac split: ACT is busy with the gelus
                        nc.vector.tensor_copy(M_sb[:, pr, 0:512], pM0[:])
                        if pr % 2 == 0:
                            nc.vector.tensor_copy(M_sb[:, pr, 512:768], pM1[:])
                        else:
                            nc.scalar.activation(M_sb[:, pr, 512:768], pM1[:], COPY)
                    pop_q(2)  # fill the M-evac -> pass-2 dependency bubble

                # ============ pass 2: y = gelu(q) @ M + b  (single GEMM) ====
                with tc.tile_pool(name="ps_y", bufs=5, space=PSUM) as ps_y:
                  for ts in range(NTS):
                    for tc4 in range(4):
                        tsl = slice(
                            ts * 512 + tc4 * 128, ts * 512 + tc4 * 128 + 128
                        )
                        py0 = ps_y.tile([128, 512], F32, tag="py")
                        py1 = ps_y.tile([128, 256], F32, tag="py", name="py1")
                        for cch in range(CCH):
                            last = cch == CCH - 1
                            nc.tensor.matmul(
                                py0[:],
                                gqT[:, cch, tsl],
                                M_sb[:, cch, 0:512],
                                start=(cch == 0),
                                stop=last,
                                skip_group_check=True,
                            )
                            nc.tensor.matmul(
                                py1[:],
                                gqT[:, cch, tsl],
                                M_sb[:, cch, 512:768],
                                start=(cch == 0),
                                stop=last,
                                skip_group_check=True,
                            )
                        y_sb = ypool.tile([128, C], F32)
                        nc.vector.tensor_add(y_sb[:, 0:512], py0[:], b_bc[:, 0:512])
                        nc.vector.tensor_add(
                            y_sb[:, 512:768], py1[:], b_bc[:, 512:768]
                        )
                        t0 = ts * 512 + tc4 * 128
                        if b == BPC - 1 and ts == NTS - 1 and tc4 >= 2:
                            # drain the last tiles on both rings in halves so
                            # transfer+receipt latencies overlap at the tail
                            nc.sync.dma_start(
                                y_d[b, t0 : t0 + 128, 0:384], y_sb[:, 0:384]
                            )
                            nc.scalar.dma_start(
                                y_d[b, t0 : t0 + 128, 384:C], y_sb[:, 384:C]
                            )
                        else:
                            # alternate the two HWDGE rings (SP / ACT) so the
                            # output queue drains twice as fast
                            eng = nc.sync if tc4 % 2 == 0 else nc.scalar
                            eng.dma_start(y_d[b, t0 : t0 + 128, :], y_sb[:])

    nc.compile()
    return nc


_cached_nc = None


def kernel(x, w_qkv, w_proj, b_proj):
    global _cached_nc
    if _cached_nc is None:
        _cached_nc = _build_program()
    nc = _cached_nc

    x = np.ascontiguousarray(x, dtype=np.float32)
    in_maps = [
        {
            "x": x[i * BPC : (i + 1) * BPC],
            "w_qkv": np.asarray(w_qkv, dtype=np.float32),
            "w_proj": np.asarray(w_proj, dtype=np.float32),
            "b_proj": np.asarray(b_proj, dtype=np.float32),
        }
        for i in range(NCORES)
    ]
    last_err = None
    for _attempt in range(3):
        try:
            res = run_bass_kernel_spmd(nc, in_maps, core_ids=list(range(NCORES)))
            out = np.concatenate(
                [res.results[i]["y"] for i in range(NCORES)], axis=0
            )
            return out.astype(np.float32)
        except Exception as e:  # transient NRT device errors recover on retry
            last_err = e
    raise last_err

